# revision 34
# baseline (speedup 1.0000x reference)
"""Trainium2 Bass kernel for nn_CrossAttention (B=2, Tq=Tk=2048, D=1024, H=16).

Sharding: 8 cores; core c owns batch b = c // 4 and query rows
[512*(c%4), 512*(c%4+1)) of that batch. Each core computes the full
attention + projections for its query slice (all 16 heads), so the
unshard is a pure concat. No collectives.

All matmuls except the output projection run in fp8e4m3 DoubleRow
(2 packed contraction rows per partition, 0.5 cycles per moving col).
Plain fp8 quantization of q/kv/W fails the 2e-2 budget through the
softmax: score noise that correlates with V via kv (q, Wq, Wk, Wv
quantization) does not average over keys. The projections therefore run
"fully compensated": W ~ W8 + Wr8 (error-feedback fp8 pair, x64 scale)
and x ~ x8 + xr8 (x8 input scale), computed as
  (W8 + Wr8)^T x8   [8 matmuls, lhsT slots (W8, Wr8), rhs slot-stride 0]
+ W8^T xr8          [4 matmuls, di-pair slots]
then descaled by 1/512 in the PSUM->SBUF copy. That leaves only
averaging-class noise (Q^T/K^T/V fp8 outputs, P fp8, Schraudolph).

Scores pack the 64-dim head contraction as [32, 2, *]: Q^T/K^T are
stored DR-banded (head h -> partition band 32*(h%4), tile g=h//4, two
free slots with dims 0..31 / 32..63) via a host-side column permutation
of Wq/Wk. P@V packs key-chunk pairs; V stationary uses stride-65
windows [64 dims | ones] so the rowsum lands in PSUM partition 64.
exp splits across ACT (true exp -> fp8) and DVE (Schraudolph int8:
byte = 8*log2(e)*scale*s + 56, bitcast to fp8e4m3), written
slot-interleaved so PV DoubleRow reads adjacent fp8 pairs.

K/V biases are free: bkk shifts all scores of a query equally
(softmax-invariant, dropped); bkv passes through the softmax average
(weights sum to 1) and folds into bo on the host.

PSUM rule found on HW: two interleaved open accumulation groups in one
2KB bank lose the first group's partials -> every bank hosts one open
chain at a time (N=512 moving makes each PV chain a single group).

Masking: fully-masked 128-chunks are dropped on the host. If any
surviving chunk is partially masked, a fallback program does per-chunk
ACT exp with a per-partition bias AP (standard mask = fast path).
"""

import os

import numpy as np
import ml_dtypes

import concourse.bass as bass
import concourse.mybir as mybir
import concourse.tile as tile
from concourse import bacc
from concourse.bass_utils import run_bass_kernel_spmd
from concourse.bass_interp import get_hw_module

B, TQ, TK, D, H = 2, 2048, 2048, 1024, 16
HD = D // H  # 64
N_CORES = 8
QLOC = (B * TQ) // N_CORES  # 512 query rows per core
SCALE = HD ** -0.5  # 0.125
WS = 64.0   # weight fp8 scale (lifts sigma~0.02 out of e4m3 subnormals)
XS = 8.0    # input fp8 scale
DESC = 1.0 / (WS * XS)

F32 = mybir.dt.float32
BF16 = mybir.dt.bfloat16
FP8 = mybir.dt.float8e4
I8 = mybir.dt.int8
Exp = mybir.ActivationFunctionType.Exp
Ident = mybir.ActivationFunctionType.Identity
Copy = mybir.ActivationFunctionType.Copy
DR = mybir.MatmulPerfMode.DoubleRow
ADD = mybir.AluOpType.add
MULT = mybir.AluOpType.mult

# Schraudolph exp constants for the fp8e4m3 bit pattern
SCH_A = float(8.0 * SCALE / np.log(2.0))
SCH_B = 56.0

_cache: dict = {}
DBG = bool(os.environ.get("KDBG"))


def _build_program(n_kc: int, fast_mask: bool, dbg: bool = False):
    NK = n_kc * 128
    n_pair = (n_kc + 1) // 2
    padded = n_pair * 2 != n_kc

    nc = bacc.Bacc("TRN2", target_bir_lowering=False, debug=False,
                   num_devices=N_CORES)

    # ---- DRAM I/O (per-core shapes) ----
    # weights interleave (W8, Wr8) pairs: [8di, 128, 2slot, D]
    qt_d = nc.dram_tensor("qt", [8, 128, QLOC], FP8, kind="ExternalInput")
    qtr_d = nc.dram_tensor("qtr", [8, 128, QLOC], FP8, kind="ExternalInput")
    kvt2_d = nc.dram_tensor("kvt2", [8, 128, 2, NK], FP8,
                            kind="ExternalInput")
    kvtr_d = nc.dram_tensor("kvtr", [8, 128, NK], FP8, kind="ExternalInput")
    wq_d = nc.dram_tensor("wq", [8, 128, 2, D], FP8, kind="ExternalInput")
    wkk_d = nc.dram_tensor("wkk", [8, 128, 2, D], FP8, kind="ExternalInput")
    wkv_d = nc.dram_tensor("wkv", [8, 128, 2, D], FP8, kind="ExternalInput")
    wo_d = nc.dram_tensor("wo", [8, 128, D], BF16, kind="ExternalInput")
    bq_d = nc.dram_tensor("bq", [8, 128], F32, kind="ExternalInput")
    bo_d = nc.dram_tensor("bo", [1, D], F32, kind="ExternalInput")
    biask_d = nc.dram_tensor("biask", [128, n_kc], F32, kind="ExternalInput")
    y_d = nc.dram_tensor("y", [QLOC, D], BF16, kind="ExternalOutput")
    if dbg:
        dbg_qtp = nc.dram_tensor("dbg_qtp", [128, 8, QLOC], I8,
                                 kind="ExternalOutput")
        dbg_kt = nc.dram_tensor("dbg_kt", [128, 8, NK], I8,
                                kind="ExternalOutput")
        dbg_ot = nc.dram_tensor("dbg_ot", [128, 8, QLOC], BF16,
                                kind="ExternalOutput")

    with tile.TileContext(nc) as tc:
        with (
            tc.tile_pool(name="const", bufs=1) as const,
            tc.tile_pool(name="persist", bufs=1) as persist,
            tc.tile_pool(name="work", bufs=4) as work,
            tc.tile_pool(name="norm", bufs=3) as norm_pool,
        ):
            # --- constants ---
            biask = const.tile([128, n_kc], F32)
            nc.sync.dma_start(biask[:], biask_d.ap())
            bq_sb = const.tile([128, 8], F32)
            nc.sync.dma_start(bq_sb[:], bq_d.ap().rearrange("c p -> p c"))
            bo_bc = const.tile([128, D], F32)
            nc.sync.dma_start(bo_bc[0:1, :], bo_d.ap())
            nc.gpsimd.partition_broadcast(bo_bc[:], bo_bc[0:1, :])

            # --- persistent activations ---
            qtp = persist.tile([128, 8, QLOC], BF16)   # Q^T (natural)
            kt = persist.tile([128, 8, NK], BF16)      # K^T (natural)
            v2 = persist.tile([128, n_pair, 2, 16 * 65 + 64], FP8)
            ot = persist.tile([128, 8, QLOC], BF16)      # O^T  [1024(m), 512]
            wo_sb = persist.tile([128, 8, D], BF16)

            v2v = v2[:, :, :, 0:1040].rearrange(
                "p k s (h c) -> p k s h c", c=65)
            if padded:
                nc.vector.memset(v2v[:, : n_pair - 1, :, :, 64:65], 1.0)
                nc.vector.memset(v2v[:, n_pair - 1, 0:1, :, 64:65], 1.0)
                nc.vector.memset(v2[:, n_pair - 1, 1:2, :], 0.0)
            else:
                nc.vector.memset(v2v[:, :, :, :, 64:65], 1.0)
            nc.vector.memset(v2[:, :, :, 1040:1104], 0.0)

            with (
                tc.tile_pool(name="wload", bufs=1) as wload,
                tc.tile_pool(name="inload", bufs=1) as inload,
                tc.tile_pool(name="ppj", bufs=2, space="PSUM") as pp_pool,
            ):
                wq_sb = wload.tile([128, 8, 2, D], FP8)
                wkk_sb = wload.tile([128, 8, 2, D], FP8)
                wkv_sb = wload.tile([128, 8, 2, D], FP8)
                qt_sb = inload.tile([128, 8, QLOC], FP8)
                qtr_sb = inload.tile([128, 8, QLOC], FP8)
                kvt2_sb = inload.tile([128, 8, 2, NK], FP8)
                kvtr_sb = inload.tile([128, 8, NK], FP8)
                # stage-A inputs on three trigger queues in parallel
                nc.sync.dma_start(qt_sb[:], qt_d.ap().rearrange(
                    "a p x -> p a x"))
                nc.scalar.dma_start(wq_sb[:], wq_d.ap().rearrange(
                    "a p s x -> p a s x"))
                nc.gpsimd.dma_start(qtr_sb[:], qtr_d.ap().rearrange(
                    "a p x -> p a x"))
                nc.sync.dma_start(kvt2_sb[:], kvt2_d.ap().rearrange(
                    "a p s x -> p a s x"))
                nc.sync.dma_start(wkk_sb[:], wkk_d.ap().rearrange(
                    "a p s x -> p a s x"))
                nc.sync.dma_start(wkv_sb[:], wkv_d.ap().rearrange(
                    "a p s x -> p a s x"))
                nc.sync.dma_start(kvtr_sb[:], kvtr_d.ap().rearrange(
                    "a p x -> p a x"))
                nc.sync.dma_start(wo_sb[:], wo_d.ap().rearrange(
                    "a p x -> p a x"))

                def proj_fullcomp(ps_ap, w_sb, rhsA, xr_sb, tslice, xs, w,
                                  xcomp=True):
                    """(W8+Wr8)^T x8 (+ W8^T xr8) into ps_ap ([128, w])."""
                    for di in range(8):
                        nc.tensor.matmul(
                            ps_ap,
                            w_sb[:, di, :, tslice],
                            rhsA(di, xs, w),
                            start=(di == 0),
                            stop=(not xcomp and di == 7), perf_mode=DR,
                        )
                    if not xcomp:
                        return
                    for dip in range(4):
                        nc.tensor.matmul(
                            ps_ap,
                            w_sb[:, 2 * dip:2 * dip + 2, 0, tslice],
                            xr_sb[:, 2 * dip:2 * dip + 2, xs:xs + w],
                            start=False, stop=(dip == 3), perf_mode=DR,
                        )

                def rhsA_qt(di, xs, w):
                    return qt_sb[:, di, xs:xs + w].unsqueeze(1) \
                        .broadcast_to([128, 2, w])

                def rhsA_kv(di, xs, w):
                    return kvt2_sb[:, di, :, xs:xs + w]

                # ---- stage A: Q^T projection ----
                for t in range(8):
                    ps = pp_pool.tile([128, QLOC], F32, tag="psq")
                    proj_fullcomp(ps[:], wq_sb, rhsA_qt, qtr_sb,
                                  slice(t * 128, t * 128 + 128), 0, QLOC)
                    nc.scalar.activation(
                        qtp[:, t, :], ps[:], Ident,
                        bias=bq_sb[:, t:t + 1], scale=DESC)

                # ---- stage B: K^T projection ----
                ksplits = [(s, min(512, NK - s)) for s in range(0, NK, 512)]
                for t in range(8):
                    for (s, w) in ksplits:
                        ps = pp_pool.tile([128, 512], F32, tag="psk")
                        proj_fullcomp(ps[:, :w], wkk_sb, rhsA_kv, kvtr_sb,
                                      slice(t * 128, t * 128 + 128), s, w,
                                      xcomp=False)
                        nc.scalar.activation(
                            kt[:, t, s:s + w], ps[:, :w],
                            Copy, bias=0.0, scale=DESC)

                # ---- stage C: V projection ----
                for kc in range(n_kc):
                    ps2 = pp_pool.tile([128, 1024], F32, tag="psv")
                    # V: lhsT = kv chunk (keys stationary), rhs = wkv
                    for dq in range(2):
                        base = dq * 512
                        for di in range(8):
                            nc.tensor.matmul(
                                ps2[:, base:base + 512],
                                kvt2_sb[:, di, :, kc * 128:kc * 128 + 128],
                                wkv_sb[:, di, :, base:base + 512],
                                start=(di == 0), stop=False, perf_mode=DR,
                            )
                        for dip in range(4):
                            nc.tensor.matmul(
                                ps2[:, base:base + 512],
                                kvtr_sb[:, 2 * dip:2 * dip + 2,
                                        kc * 128:kc * 128 + 128],
                                wkv_sb[:, 2 * dip:2 * dip + 2, 0,
                                       base:base + 512],
                                start=False, stop=(dip == 3), perf_mode=DR,
                            )
                    v2t = v2v[:, kc // 2, kc % 2, :, 0:64]
                    ps2v = ps2[:].rearrange("p (h c) -> p h c", c=64)
                    if kc % 2 == 0:
                        nc.vector.tensor_scalar_mul(v2t, ps2v, DESC)
                    else:
                        nc.scalar.activation(v2t, ps2v, Copy,
                                             bias=0.0, scale=DESC)

            # ---- stage D: attention ----
            pa_ctx = tc.tile_pool(name="pat", bufs=1, space="PSUM")
            pa_pool = pa_ctx.__enter__()
            # Software-pipelined: engine queues are FIFO, so a PV
            # waiting on its exp blocks every later PE instruction. Emit
            # PV(pc-2) after scores(pc) so exp has ~1.1us to land, and
            # defer each head's normalize chain (DVE recip -> DMA hop ->
            # Pool bcast -> DVE mult) into the next head's loop so it
            # never stalls the DVE exp stream.
            PIPE = 4
            pend_norm = []

            def emit_norm(po_h, h_):
                rb = norm_pool.tile([128, QLOC], F32, tag="rb")
                nc.vector.reciprocal(rb[64:65, :], po_h[64:65, :])
                rs0 = norm_pool.tile([1, QLOC], F32, tag="rs0")
                nc.sync.dma_start(rs0[:], rb[64:65, :])
                nc.gpsimd.partition_broadcast(rb[0:64, :], rs0[0:1, :],
                                              channels=64)
                nc.vector.tensor_tensor(
                    out=ot[64 * (h_ % 2):64 * (h_ % 2) + 64, h_ // 2, :],
                    in0=po_h[0:64, :], in1=rb[0:64, :], op=MULT)

            for h in range(H):
                hc, r0h = h // 2, 64 * (h % 2)
                po = pa_pool.tile([128, QLOC], F32, tag="po", bufs=2,
                                  name=f"po_{h}")
                pts = []
                for pc in range(n_pair):
                    pss = pa_pool.tile([128, 1024], F32, tag="pss", bufs=3)
                    n_in = 2 if (2 * pc + 1 < n_kc) else 1
                    for kc_in in range(n_in):
                        kc = 2 * pc + kc_in
                        nc.tensor.matmul(
                            pss[:, kc_in * 512:kc_in * 512 + 512],
                            kt[r0h:r0h + 64, hc, kc * 128:kc * 128 + 128],
                            qtp[r0h:r0h + 64, hc, :],
                            start=True, stop=True,
                            tile_position=(r0h, 0),
                        )
                    pt2 = work.tile([128, 1024], FP8, tag="pt2",
                                    bufs=PIPE + 3)
                    pt_v = pt2[:].rearrange("p (q two) -> p two q", two=2)
                    ps_v = pss[:].rearrange("p (two q) -> p two q", two=2)
                    if fast_mask:
                        idx = (h * n_pair + pc) % 5
                        if n_in == 1:
                            ps_in, pt_out = ps_v[:, 0:1, :], pt_v[:, 0:1, :]
                        else:
                            ps_in, pt_out = ps_v, pt_v
                        if idx in (0, 2, 4):
                            nc.scalar.activation(pt_out, ps_in, Exp,
                                                 bias=0.0, scale=SCALE)
                        else:
                            nc.vector.tensor_scalar(
                                pt_out.bitcast(I8), ps_in, SCH_A, SCH_B,
                                MULT, ADD)
                    else:
                        for kc_in in range(n_in):
                            kc = 2 * pc + kc_in
                            nc.scalar.activation(
                                pt_v[:, kc_in, :], ps_v[:, kc_in, :], Exp,
                                bias=biask[:, kc:kc + 1], scale=SCALE)
                    pts.append(pt_v)
                    if pc == 1 and pend_norm:
                        emit_norm(*pend_norm.pop(0))
                    if pc >= PIPE:
                        pv = pc - PIPE
                        nc.tensor.matmul(
                            po[:],
                            v2[:, pv, :, 65 * h:65 * h + 128],
                            pts[pv][:],
                            start=(pv == 0), stop=(pv == n_pair - 1),
                            perf_mode=DR,
                        )
                for pv in range(max(0, n_pair - PIPE), n_pair):
                    nc.tensor.matmul(
                        po[:],
                        v2[:, pv, :, 65 * h:65 * h + 128],
                        pts[pv][:],
                        start=(pv == 0), stop=(pv == n_pair - 1),
                        perf_mode=DR,
                    )
                pend_norm.append((po, h))
            while pend_norm:
                emit_norm(*pend_norm.pop(0))

            if dbg:
                nc.sync.dma_start(dbg_ot.ap(), ot[:])

            # ---- stage E: output projection (bf16, bias via ones-row) ----
            for qm in range(QLOC // 128):
                y_sb = work.tile([128, D], BF16, tag="y", bufs=2)
                for nn in range(2):
                    pso = pa_pool.tile([128, 1024], F32, tag="pss", bufs=3)
                    for mc in range(8):
                        nc.tensor.matmul(
                            pso[:, 0:512],
                            ot[:, mc, qm * 128:qm * 128 + 128],
                            wo_sb[:, mc, nn * 512:nn * 512 + 512],
                            start=(mc == 0), stop=(mc == 7),
                        )
                    nc.vector.tensor_tensor(
                        out=y_sb[:, nn * 512:nn * 512 + 512], in0=pso[:, 0:512],
                        in1=bo_bc[:, nn * 512:nn * 512 + 512], op=ADD)
                nc.sync.dma_start(y_d.ap()[qm * 128:qm * 128 + 128, :],
                                  y_sb[:])
            pa_ctx.__exit__(None, None, None)

    nc.compile()
    nc.m = get_hw_module(nc.m)
    return nc


def _get_program(n_kc: int, fast_mask: bool):
    key = (n_kc, fast_mask)
    if key not in _cache:
        _cache[key] = _build_program(n_kc, fast_mask, dbg=DBG)
    return _cache[key]


_F8 = ml_dtypes.float8_e4m3


def _f8pair(x, s):
    """Error-feedback fp8 pair of x*s: returns (x8, xr8) float8 arrays."""
    xs = np.ascontiguousarray(x, dtype=np.float32) * np.float32(s)
    x8 = xs.astype(_F8)
    xr8 = (xs - x8.astype(np.float32)).astype(_F8)
    return x8, xr8


def _dr_perm():
    """Column permutation for the DR-banded Q/K layouts."""
    t = np.arange(8)[:, None]
    p = np.arange(128)[None, :]
    g, s = t // 2, t % 2
    return (64 * (4 * g + p // 32) + 32 * s + (p % 32)).reshape(-1)


def kernel(q, kv, key_padding_mask, Wq, bq, Wkv, bkv, Wo, bo):
    q = np.asarray(q, dtype=np.float32)
    kv = np.asarray(kv, dtype=np.float32)
    mask = np.asarray(key_padding_mask).astype(bool)
    Wq = np.asarray(Wq, dtype=np.float32)
    bq = np.asarray(bq, dtype=np.float32)
    Wkv = np.asarray(Wkv, dtype=np.float32)
    bkv = np.asarray(bkv, dtype=np.float32)
    Wo = np.asarray(Wo, dtype=np.float32)
    bo = np.asarray(bo, dtype=np.float32)

    live = ~mask
    chunk_live = live.reshape(B, TK // 128, 128).any(axis=2).any(axis=0)
    active = np.flatnonzero(chunk_live)
    n_kc = int(len(active))
    assert n_kc >= 1
    NK = n_kc * 128
    sel = (active[:, None] * 128 + np.arange(128)[None, :]).reshape(-1)
    fast_mask = bool(live[:, sel].all())

    nc = _get_program(n_kc, fast_mask)

    def wpack(Wcols):
        w8, wr8 = _f8pair(Wcols, WS)
        out = np.empty((8, 128, 2, D), _F8)
        out[:, :, 0, :] = w8.reshape(8, 128, D)
        out[:, :, 1, :] = wr8.reshape(8, 128, D)
        return out

    wq_h = wpack(Wq)
    wkk_h = wpack(Wkv[:, :D])
    wkv_h = wpack(Wkv[:, D:])
    wo_h = np.ascontiguousarray(Wo).astype(ml_dtypes.bfloat16).reshape(
        8, 128, D)
    bq_h = np.ascontiguousarray(bq).reshape(8, 128)
    # bkk is softmax-invariant; bkv folds through the softmax average
    bo_h = (bo + bkv[D:] @ Wo).astype(np.float32).reshape(1, D)

    shared = {
        "wq": wq_h, "wkk": wkk_h, "wkv": wkv_h, "wo": wo_h,
        "bq": bq_h, "bo": bo_h,
    }

    in_maps = []
    for c in range(N_CORES):
        b = c // 4
        r0 = (c % 4) * QLOC
        qt8, qtr8 = _f8pair(q[b, r0:r0 + QLOC, :].T, XS)
        kvt8, kvtr8 = _f8pair(kv[b][sel, :].T, XS)
        bias_flat = np.where(mask[b][sel], np.float32(-80.0), np.float32(0.0))
        biask = np.ascontiguousarray(
            bias_flat.reshape(n_kc, 128).T).astype(np.float32)
        m = dict(shared)
        kvt2 = np.empty((8, 128, 2, NK), _F8)
        kvt2[:, :, 0, :] = kvt8.reshape(8, 128, NK)
        kvt2[:, :, 1, :] = kvt8.reshape(8, 128, NK)
        m.update({
            "qt": qt8.reshape(8, 128, QLOC),
            "qtr": qtr8.reshape(8, 128, QLOC),
            "kvt2": kvt2,
            "kvtr": kvtr8.reshape(8, 128, NK),
            "biask": biask,
        })
        in_maps.append(m)

    res = run_bass_kernel_spmd(
        nc, in_maps, core_ids=list(range(N_CORES)), trace=False)

    out = np.empty((B, TQ, D), dtype=np.float32)
    for c in range(N_CORES):
        b = c // 4
        r0 = (c % 4) * QLOC
        out[b, r0:r0 + QLOC, :] = res.results[c]["y"].astype(np.float32)
    return out
